# revision 25
# baseline (speedup 1.0000x reference)
"""Channel-attention module kernel for Trainium2 (8 NeuronCores, data parallel).

Computes, per batch b:
    flat   = x[b].reshape(C, H*W)
    scores = flat @ flat.T                       # [C, C]
    attn   = softmax(scores, axis=-1)
    attn   = max(attn, -1, keepdims) - attn
    e      = attn.T @ flat                       # [C, H*W]
    out[b] = x[b] + beta * e

Key identity used: with m = rowmax(scores), S = sum(exp(scores - m)),
    rowmax(softmax) - softmax = (1 - exp(scores - m)) / S
so attn (with beta folded in) = beta/S - (beta/S) * exp(scores - m).

The whole kernel runs in bf16: the host rounds x to bf16 (RNE) before
upload, halving input-side HBM traffic vs fp32 (the matmul path consumed
only the high 16 bits anyway), and the output is written bf16 (host
upcasts after gather). Worst-case round-off ~2^-8 relative, well inside
the 2e-2 gate. The residual is folded into the attention matrix
(x + attn.T @ x == (attn + I).T @ x), so stage 2 is pure matmul +
PSUM->SBUF copy.

Scheduling notes (hard-won):
  * Each engine's work is kept in long stall-free runs: PE stalls reset
    its p-state ramp (0.65 -> 1.2 -> 2.4 GHz only while continuously
    busy), so a fine-grained cross-stage interleave makes every matmul
    ~2x slower.  Stage 1 of batch b+1 rides inside stage 2 of batch b
    at CHUNK granularity (~4us PE sub-phases) -- coarse enough to hold
    the p-state, fine enough to hide S2's PSUM-copy latency under S1's
    PE work.  Dummy identity-transposes warm the PE during the DMA
    lead-in and keep it hot across the last batch's softmax.
  * The DMA ring (one shared FIFO for input + output) sustains ~420 GB/s
    when fed; inputs are triple-buffered (batches 0-2 up front, batch 3
    at softmax(1)) and out-DMAs are enqueued at production, so the back
    half is production-limited, not ring-limited.
  * The softmax chain is emitted before the lookahead transposes so its
    ops head the in-order vector/scalar queues.
  * PSUM: 3 transpose banks + 1 scores bank + 4 stage-2 banks (E_TILE
    512: 4-deep copy slack keeps the PE from stalling in the drain).

Sharding: batch dim (32) split over 8 cores, 4 batches per core, beta
replicated; no cross-core communication.
"""

import ml_dtypes
import numpy as np

import concourse.bass as bass
import concourse.mybir as mybir
import concourse.tile as tile
from concourse import bacc
from concourse.bass_utils import run_bass_kernel_spmd
from concourse.masks import make_identity

N_CORES = 8
B_TOTAL, C, H, W = 32, 128, 128, 128
HW = H * W                      # 16384
B_LOCAL = B_TOTAL // N_CORES    # 4
P = 128

F32 = mybir.dt.float32
BF16 = mybir.dt.bfloat16

MM_N = 512                      # stage-2 matmul free dim (one PSUM bank fp32)
E_TILE = 512                    # stage-2 psum tile (1 bank, 1 matmul, 1 copy)
TG = 8                          # transposed 128-chunks per bf16 PSUM bank
OUT_CHUNK = 4096                # output staging chunk (8 KB/partition, 1 MB DMA)
IN_CHUNK = 4096                 # input DMA chunk (1 MB DMA)
LOOKAHEAD = 4                   # transposed groups of b+1 emitted pre-S2(b)
MM_LAG = 3                      # scores matmul group lag behind transposes
WARMUP = 40                     # dummy PE transposes during the DMA lead-in


def build_bass(b_local: int = B_LOCAL) -> bass.Bass:
    nc = bacc.Bacc("TRN2", target_bir_lowering=False)
    x = nc.dram_tensor("x", [b_local, C, HW], BF16, kind="ExternalInput")
    beta = nc.dram_tensor("beta", [1], F32, kind="ExternalInput")
    out = nc.dram_tensor("out", [b_local, C, HW], BF16, kind="ExternalOutput")

    n_chunk = HW // P           # 128 transposed chunks per batch
    n_group = n_chunk // TG     # 16
    n_out = HW // OUT_CHUNK     # 4 output chunks per batch
    e_per_out = OUT_CHUNK // E_TILE
    mm_per_e = E_TILE // MM_N
    n_quarter = HW // IN_CHUNK  # 4 input quarters per batch

    assert OUT_CHUNK == IN_CHUNK

    with tile.TileContext(nc) as tc:
        with (
            tc.tile_pool(name="singles", bufs=1) as singles,
            tc.tile_pool(name="flats", bufs=3 * n_quarter) as flats,
            tc.tile_pool(name="ats", bufs=4 + LOOKAHEAD) as ats,
            tc.tile_pool(name="outs", bufs=8) as outs,
            tc.tile_pool(name="sm", bufs=2) as sm,
            tc.tile_pool(name="ps_t", bufs=3, space="PSUM") as ps_t,
            tc.tile_pool(name="ps_s", bufs=1, space="PSUM") as ps_s,
            tc.tile_pool(name="ps_e", bufs=4, space="PSUM") as ps_e,
        ):
            ident = singles.tile([P, P], BF16)
            beta_b = singles.tile([P, 1], F32)
            negbeta_b = singles.tile([P, 1], F32)

            flat_tiles: dict[tuple[int, int], bass.AP] = {}
            at_tiles: dict[tuple[int, int], bass.AP] = {}
            scores_tiles: dict[int, bass.AP] = {}
            g_per_q = IN_CHUNK // (TG * P)  # transpose groups per quarter

            # PSUM->SBUF copies are the serial tax of both pipeline stages
            # (~1us each on a single engine); split them between vector and
            # scalar by greedy least-finish-time using measured per-copy
            # costs (DVE: 0.69us bf16 / 1.22us f32-cast; ACT: 1.03us both;
            # gpsimd has no PSUM port so only these two can drain PSUM).
            copy_load = [0.0, 0.0]  # projected busy us: [vector, scalar]
            COPY_COST = {  # (engine, is_f32_source) -> us per tile
                (0, False): 0.69, (0, True): 0.66,
                (1, False): 1.06, (1, True): 0.64,
            }
            copy_fns = [
                lambda o, i: nc.vector.tensor_copy(out=o, in_=i),
                lambda o, i: nc.scalar.copy(out=o, in_=i),
            ]

            def emit_copy(o, i, avoid_scalar=False, f32_src=False):
                if avoid_scalar:
                    k = 0
                else:
                    k = 0 if (
                        copy_load[0] + COPY_COST[(0, f32_src)]
                        <= copy_load[1] + COPY_COST[(1, f32_src)]
                    ) else 1
                copy_load[k] += COPY_COST[(k, f32_src)]
                copy_fns[k](o, i)

            def emit_in_quarter(b, q, split=1):
                # One SBUF tile per (batch, quarter); triple-buffered over
                # batches so the input stream runs well ahead of compute.
                t = flats.tile([P, IN_CHUNK], BF16, tag="flat", name=f"fl{b}_{q}")
                flat_tiles[(b, q)] = t
                base = q * IN_CHUNK
                step = IN_CHUNK // split
                for s in range(split):
                    sl = slice(base + s * step, base + (s + 1) * step)
                    nc.sync.dma_start(
                        out=t[:, s * step : (s + 1) * step], in_=x[b, :, sl]
                    )

            def emit_t_group(b, g, avoid_scalar=False):
                fq = flat_tiles[(b, g // g_per_q)]
                base = (g % g_per_q) * TG * P
                tp = ps_t.tile([P, TG * P], BF16, tag="tp")
                for jj in range(TG):
                    nc.tensor.transpose(
                        tp[:, jj * P : (jj + 1) * P],
                        fq[:, base + jj * P : base + (jj + 1) * P],
                        ident,
                    )
                at = ats.tile([P, TG * P], BF16, tag="at")
                emit_copy(at, tp, avoid_scalar=avoid_scalar)
                at_tiles[(b, g)] = at

            def emit_m_group(b, g):
                if g == 0:
                    scores_tiles[b] = ps_s.tile(
                        [P, P], F32, tag="scores", name=f"scores{b}"
                    )
                scores_ps = scores_tiles[b]
                at = at_tiles.pop((b, g))
                for jj in range(TG):
                    k = g * TG + jj
                    nc.tensor.matmul(
                        scores_ps,
                        at[:, jj * P : (jj + 1) * P],
                        at[:, jj * P : (jj + 1) * P],
                        start=(k == 0),
                        stop=(k == n_chunk - 1),
                    )

            # input: first quarter split fine so the first transposes
            # start ~2 us after the stream opens; batches 0-2 up front.
            emit_in_quarter(0, 0, split=4)
            emit_in_quarter(0, 1, split=2)
            for q in range(2, n_quarter):
                emit_in_quarter(0, q)
            for b in range(1, min(3, b_local)):
                for q in range(n_quarter):
                    emit_in_quarter(b, q)

            # setup AFTER the input DMAs so they head the sync queue
            make_identity(nc, ident)
            bap = beta[:]
            beta_bcast = bass.AP(
                tensor=bap.tensor, offset=bap.offset, ap=[[0, P], [1, 1]]
            )
            nc.gpsimd.dma_start(out=beta_b, in_=beta_bcast)
            nc.vector.tensor_scalar_mul(negbeta_b, beta_b, -1.0)

            # Warm the PE p-state during the DMA lead-in: dummy transposes
            # of the identity keep the tensor engine continuously executing
            # so the real transposes start at full clock.
            for w in range(WARMUP // TG):
                wtp = ps_t.tile([P, TG * P], BF16, tag="tp")
                for jj in range(TG):
                    nc.tensor.transpose(
                        wtp[:, jj * P : (jj + 1) * P], ident, ident
                    )

            def s1_step(b1, g):
                if g < n_group:
                    emit_t_group(b1, g)
                if 0 <= g - MM_LAG < n_group:
                    emit_m_group(b1, g - MM_LAG)

            # ---- stage 1 of batch 0 (nothing to interleave against) ----
            for g in range(n_group + MM_LAG):
                s1_step(0, g)

            for b in range(b_local):
                # ---- softmax transform: attn = beta/S - (beta/S)*exp(s-m) ----
                # Emitted BEFORE the lookahead transposes: the chain's ops
                # must head the vector/scalar queues, or the lookahead
                # at-copies delay reduce_max/exp by ~1us each (in-order
                # engine queues) and the PE stalls that much longer on attn.
                scores_ps = scores_tiles.pop(b)
                neg_max = sm.tile([P, 1], F32, tag="neg_max")
                nc.vector.reduce_max(
                    out=neg_max,
                    in_=scores_ps,
                    axis=mybir.AxisListType.X,
                    negate=True,
                )
                ex = sm.tile([P, P], F32, tag="ex")
                sumexp = sm.tile([P, 1], F32, tag="sumexp")
                # accum_out fuses the row-sum into the EXP pass (one
                # cross-engine hop shorter than a separate reduce_sum).
                nc.scalar.activation(
                    out=ex,
                    in_=scores_ps,
                    func=mybir.ActivationFunctionType.Exp,
                    bias=neg_max,
                    scale=1.0,
                    accum_out=sumexp,
                )
                r = sm.tile([P, 1], F32, tag="r")
                nc.vector.reciprocal(r, sumexp)
                rb = sm.tile([P, 1], F32, tag="rb")
                nc.vector.tensor_mul(rb, r, beta_b)
                nrb = sm.tile([P, 1], F32, tag="nrb")
                nc.vector.tensor_mul(nrb, r, negbeta_b)
                attn0 = sm.tile([P, P], BF16, tag="attn0")
                # out = Identity(ex * nrb + rb) = rb - rb*ex
                nc.scalar.activation(
                    out=attn0,
                    in_=ex,
                    func=mybir.ActivationFunctionType.Identity,
                    bias=rb,
                    scale=nrb,
                )
                # Fold the residual into the attention matrix:
                # x + attn.T @ x == (attn + I).T @ x, so the PE array does
                # the residual add and stage 2 needs no vector adds.
                attn = sm.tile([P, P], BF16, tag="attn")
                nc.vector.tensor_add(out=attn, in0=attn0, in1=ident)

                # lookahead transposes of b+1: PE work to chew on while the
                # softmax chain runs on vector/scalar.  For the last batch
                # there is no b+1; dummy transposes keep the PE p-state
                # ramped across the softmax latency instead.
                last = b == b_local - 1
                if not last:
                    for g in range(LOOKAHEAD):
                        s1_step(b + 1, g)
                else:
                    for w in range(6):
                        wtp = ps_t.tile([P, TG * P], BF16, tag="tp")
                        for jj in range(TG):
                            nc.tensor.transpose(
                                wtp[:, jj * P : (jj + 1) * P], ident, ident
                            )

                # batch 3's input: enqueued here (after softmax(1)) so the
                # ring FIFO reaches it while stage 2 of b=1 still computes.
                if b == 1 and b_local > 3:
                    for q in range(n_quarter):
                        emit_in_quarter(3, q)

                # ---- stage 2: e = attn.T @ x (bf16), out = x + e ----
                # The rest of S1(b+1) rides along at chunk granularity
                # (~4us PE sub-phases: coarse enough to hold the PE p-state,
                # fine enough that S2's copy latency hides under S1's PE
                # work instead of pacing the kernel).
                g_rem = n_group + MM_LAG - LOOKAHEAD  # S1 steps left for b+1
                for jo in range(n_out):
                    fq = flat_tiles.pop((b, jo))
                    oc = outs.tile([P, OUT_CHUNK], BF16, tag="oc")
                    for je in range(e_per_out):
                        e_ps = ps_e.tile([P, E_TILE], F32, tag="e")
                        for jm in range(mm_per_e):
                            lo = (je * mm_per_e + jm) * MM_N
                            nc.tensor.matmul(
                                e_ps[:, jm * MM_N : (jm + 1) * MM_N],
                                attn,
                                fq[:, lo : lo + MM_N],
                                start=True,
                                stop=True,
                            )
                        emit_copy(
                            oc[:, je * E_TILE : (je + 1) * E_TILE],
                            e_ps,
                            f32_src=True,
                        )
                    if not last:
                        n_steps = g_rem // (n_out - jo)
                        g_rem -= n_steps
                        g0 = n_group + MM_LAG - LOOKAHEAD - g_rem - n_steps
                        for j in range(n_steps):
                            s1_step(b + 1, LOOKAHEAD + g0 + j)
                    base = jo * OUT_CHUNK
                    # the back half of the kernel is production-limited, so
                    # out-DMAs are enqueued as soon as each chunk's copies
                    # are emitted; the last batch drains in fine splits so
                    # the stream tracks the copies closely.
                    n_split = 4 if last else 1
                    step = OUT_CHUNK // n_split
                    for s in range(n_split):
                        nc.sync.dma_start(
                            out=out[b, :, base + s * step : base + (s + 1) * step],
                            in_=oc[:, s * step : (s + 1) * step],
                        )
    nc.compile()
    return nc


_NC_CACHE: dict[int, bass.Bass] = {}


def _get_nc(b_local: int = B_LOCAL) -> bass.Bass:
    if b_local not in _NC_CACHE:
        _NC_CACHE[b_local] = build_bass(b_local)
    return _NC_CACHE[b_local]


def _run(x: np.ndarray, beta: np.ndarray, trace: bool = False):
    beta = np.ascontiguousarray(np.asarray(beta), dtype=np.float32).reshape(1)
    # Round x to bf16 on the host (RNE): the device matmul path is bf16
    # anyway, and shipping 2-byte words halves input-side HBM traffic.
    xr = np.asarray(x, dtype=np.float32).reshape(B_TOTAL, C, HW)
    xr = xr.astype(ml_dtypes.bfloat16)
    in_maps = []
    for i in range(N_CORES):
        shard = np.ascontiguousarray(xr[i * B_LOCAL : (i + 1) * B_LOCAL])
        in_maps.append({"x": shard, "beta": beta})
    nc = _get_nc()
    res = run_bass_kernel_spmd(
        nc, in_maps, core_ids=list(range(N_CORES)), trace=trace
    )
    parts = [np.asarray(res.results[i]["out"]) for i in range(N_CORES)]
    full = np.concatenate(parts, axis=0).reshape(B_TOTAL, C, H, W)
    return np.ascontiguousarray(full.astype(np.float32)), res


def kernel(x: np.ndarray, beta: np.ndarray) -> np.ndarray:
    out, _ = _run(x, beta, trace=False)
    return out


def kernel_traced(x: np.ndarray, beta: np.ndarray):
    """Like kernel() but also returns the BassKernelResults (with profile)."""
    return _run(x, beta, trace=True)
